# revision 40
# baseline (speedup 1.0000x reference)
"""Trainium2 Bass kernel for nn_AddMaskHead (ROI mask head: bilinear pool +
concat + conv3x3 + BN + ReLU).

Self-contained: hardcodes shapes B=2, N=256 (512 boxes), C=256, H=96, W=128,
P=14. Shards data-parallel over the 512 boxes across 8 NeuronCores (64
boxes/core; each core's boxes all come from a single image).

Conv strategy: 2-D Winograd F(2x2, 3x3). Each 14x14 output splits into 7x7
tiles of 2x2; per tile the conv needs 16 (u,v) multiply positions instead of
36 (4 muls/output vs 9 direct, vs 6 for the previous 1-D scheme). The mask
half of the concat input is Winograd-transformed on the host (static linear
map, shipped as bf16 -> same DMA bytes as the raw f32 mask). The ROI-pooled
half is folded through per-box separable interpolation matrices: UC2 =
(upsampled concat feats x transformed weights) is box-independent and built
once on the PE; per box only the rank-1 frame GW2 = MY' (x) MX' is built on
the DVE (bf16, n-innermost so every op hits the packed 2-byte fast path).
Matmuls accumulate 4 K=128 chunks into one 392-column PSUM bank per (u,v)
position (8 boxes x 49 tiles), uv-major so banks recycle through a
scalar/gpsimd evacuation pipeline while the PE streams on.
"""

import sys, os, types

sys.path.insert(0, "/opt/trn_rl_repo")

import numpy as np
import concourse.bass as bass
import concourse.mybir as mybir
import concourse.tile as tile
from concourse import bacc
from concourse.masks import make_identity

F32 = mybir.dt.float32
BF16 = mybir.dt.bfloat16
I32 = mybir.dt.int32
ALU = mybir.AluOpType
AF = mybir.ActivationFunctionType

N_CORES = 8
NB = 64            # boxes per core
BATCH = 8          # boxes per inner batch
NBATCH = NB // BATCH
P = 14             # pooler resolution
C = 256            # channels
H, W = 96, 128     # feature map
PQ = P * P         # 196
Q1 = PQ - 128      # 68 (q-chunk 1 size)
T7 = 7             # winograd tiles per axis
NT = T7 * T7       # 49 tiles
UV = 16            # winograd multiply positions per tile
XW2SZ = 2 * UV * NT * BATCH   # per-batch transformed-mask elems per partition


def _axis_static(in_s, out_s=P):
    # mirrors reference._resize_bilinear axis() in exact f32 arithmetic
    s = (np.arange(out_s, dtype=np.float32) + np.float32(0.5)) * np.float32(in_s / out_s) - np.float32(0.5)
    s = np.maximum(s, np.float32(0.0))
    i0 = np.minimum(np.floor(s).astype(np.int32), in_s - 1)
    i1 = np.minimum(i0 + 1, in_s - 1)
    w = (s - i0.astype(np.float32)).astype(np.float32)
    return i0, i1, w


YS0, YS1, WYS = _axis_static(H)
XS0, XS1, WXS = _axis_static(W)

BT = np.array([[1, 0, -1, 0], [0, 1, 1, 0], [0, -1, 1, 0], [0, 1, 0, -1]],
              np.float32)
GM = np.array([[1, 0, 0], [.5, .5, .5], [.5, -.5, .5], [0, 0, 1]], np.float32)


def _consts_p():
    # per-partition constants: [128, 4] = (yv_q0, xv_q0, yv_q1, xv_q1); -1 pads
    arr = np.full((128, 4), -1.0, dtype=np.float32)
    for p in range(128):
        arr[p, 0] = (p // P)
        arr[p, 1] = (p % P)
    for p in range(Q1):
        q = 128 + p
        arr[p, 2] = (q // P)
        arr[p, 3] = (q % P)
    return arr


def _consts_f():
    # free-dim constants (broadcast to all partitions on device):
    # [0:14] jc = arange(14)+0.5; [14:28] wys; [28:42] 1-wys;
    # [48:62] wxs; [62:76] 1-wxs
    arr = np.zeros((1, 80), dtype=np.float32)
    arr[0, 0:14] = np.arange(P, dtype=np.float32) + np.float32(0.5)
    arr[0, 14:28] = WYS
    arr[0, 28:42] = np.float32(1.0) - WYS
    arr[0, 48:62] = WXS
    arr[0, 62:76] = np.float32(1.0) - WXS
    return arr


def build_kernel():
    nc = bacc.Bacc(None)

    featsl = nc.declare_dram_parameter("featsl", [128, 2 * P * 2 * P * 2], F32,
                                       isOutput=False)
    boxes = nc.declare_dram_parameter("boxes", [NB, 4], F32, isOutput=False)
    # host 2-D-winograd mask features [cp, batch, ci, uv, ty, tx, n]
    xwm_d = nc.declare_dram_parameter("xwm", [128, NBATCH * XW2SZ], BF16,
                                      isOutput=False)
    wt_d = nc.declare_dram_parameter("wt", [128, 4, UV, 256], BF16, isOutput=False)
    epi_d = nc.declare_dram_parameter("epi", [128, 5, 2], F32, isOutput=False)
    cp_d = nc.declare_dram_parameter("consts_p", [128, 4], F32, isOutput=False)
    cf_d = nc.declare_dram_parameter("consts_f", [1, 80], F32, isOutput=False)
    out_d = nc.declare_dram_parameter("out", [NB, C, P, P], F32, isOutput=True)

    out_v = out_d.rearrange("n (oh op) i j -> op oh n (i j)", op=128)

    RC14 = float(np.float32(1.0) / np.float32(P))

    with tile.TileContext(nc) as tc:
        with tc.tile_pool(name="persist", bufs=1) as pp:
            # ---------- persistent tiles ----------
            # mask-half transformed weights [ci_par, ci_hi, uv, o]
            Wm = pp.tile([128, 2, UV, 256], BF16, tag="Wm")
            # UC2[qc][q, uv, o] = sum_ci cf[ci, q] * Wc2[uv][ci, o]
            UC = [pp.tile([128, UV, 256], BF16, tag=f"UC{qc}", name=f"UC{qc}")
                  for qc in range(2)]
            # host-transformed mask features per batch [cp, ci, uv, ty, tx, n]
            XWb = [pp.tile([128, 2, UV, T7, T7, BATCH], BF16, tag=f"xw{i}",
                           name=f"xw{i}") for i in range(2)]
            # per-box winograd interp frames per q-chunk (double-buffered)
            GWb = [[pp.tile([128, UV, T7, T7, BATCH], BF16, tag=f"gw{qc}{i}",
                            name=f"gw{qc}{i}") for i in range(2)]
                   for qc in range(2)]
            cpt = pp.tile([128, 4], F32, tag="cpt")
            cft = pp.tile([128, 80], F32, tag="cft")
            epi = pp.tile([128, 5, 2], F32, tag="epi")
            scale_e = pp.tile([128, 2], F32, tag="scale_e")
            bias_e = pp.tile([128, 2], F32, tag="bias_e")
            # per-box interpolation data for all 64 boxes (bf16: indices are
            # small ints = exact; lerp weights lose ~2^-9 = fine at 2e-2)
            BW = 28  # box_math batch width (boxes per call after batch 0)
            Y0 = pp.tile([128, NB, P], BF16, tag="Y0")
            Y1 = pp.tile([128, NB, P], BF16, tag="Y1")
            WY = pp.tile([128, NB, P], BF16, tag="WY")
            OWY = pp.tile([128, NB, P], BF16, tag="OWY")
            X0 = pp.tile([128, NB, P], BF16, tag="X0")
            X1 = pp.tile([128, NB, P], BF16, tag="X1")
            WX = pp.tile([128, NB, P], BF16, tag="WX")
            OWX = pp.tile([128, NB, P], BF16, tag="OWX")
            # padded interp row/col weights [q, qc, 16, n] (rows 0/15 zero)
            myp = pp.tile([128, 2, 16, BATCH], F32, tag="myp")
            mxp = pp.tile([128, 2, 16, BATCH], F32, tag="mxp")
            MYt = pp.tile([128, 2, 4, T7, BATCH], BF16, tag="MYt")
            MXt = pp.tile([128, 2, 4, T7, BATCH], BF16, tag="MXt")
            # box-math temps (BW-wide; the 8-box phase-0 call uses a slice)
            bxb = pp.tile([128, NB, 4], F32, tag="bxb")
            abx = pp.tile([128, BW, 4], F32, tag="abx")
            bm_i4 = pp.tile([128, BW, 4], I32, tag="bm_i4")
            bm_f4 = pp.tile([128, BW, 4], F32, tag="bm_f4")
            bm_s = pp.tile([128, BW, P], F32, tag="bm_s")
            bm_f = pp.tile([128, BW, P], F32, tag="bm_f")
            bm_i = pp.tile([128, BW, P], I32, tag="bm_i")
            bm_a = pp.tile([128, BW, P], F32, tag="bm_a")
            bm_d = pp.tile([128, BW], F32, tag="bm_d")
            bm_n = pp.tile([128, BW], F32, tag="bm_n")
            bm_q = pp.tile([128, BW], F32, tag="bm_q")
            bm_h = pp.tile([128, BW], F32, tag="bm_h")
            cfv = pp.tile([128, 2, P, P], F32, tag="cfv")
            cfb = pp.tile([128, 2, P, P], BF16, tag="cfb")
            cmpb = pp.tile([128, 2, P, BATCH], F32, tag="cmpb")

            jc_b = cft[:, 0:14]

            def g_build(b):
                """interp frames GW2[qc][uv, ty, tx, n] for batch b from box
                data rows [8b, 8b+8): my/mx (DVE smalls), winograd row combos
                MY'/MX', then the rank-1 outer product in bf16 fast mode
                (split DVE / gpsimd)."""
                slot = b % 2
                ns = slice(b * BATCH, (b + 1) * BATCH)
                sh = [128, 2, P, BATCH]
                yv = cpt[:, 0:4:2, None, None].to_broadcast(sh)
                xv = cpt[:, 1:4:2, None, None].to_broadcast(sh)

                def tr(a):
                    # transposed read of a box array: [r, n] from [n, r]
                    v = a[:, ns]  # [128, BATCH, P]
                    return v.transpose([0, 2, 1])[:, None].to_broadcast(sh)

                my = myp[:, :, 1:15, :]
                mx = mxp[:, :, 1:15, :]
                nc.vector.tensor_tensor(my, tr(Y0), yv, ALU.is_equal)
                nc.vector.tensor_mul(my, my, tr(OWY))
                nc.vector.tensor_tensor(cmpb[:], tr(Y1), yv, ALU.is_equal)
                nc.vector.tensor_mul(cmpb[:], cmpb[:], tr(WY))
                nc.vector.tensor_add(my, my, cmpb[:])
                nc.vector.tensor_tensor(mx, tr(X0), xv, ALU.is_equal)
                nc.vector.tensor_mul(mx, mx, tr(OWX))
                nc.vector.tensor_tensor(cmpb[:], tr(X1), xv, ALU.is_equal)
                nc.vector.tensor_mul(cmpb[:], cmpb[:], tr(WX))
                nc.vector.tensor_add(mx, mx, cmpb[:])
                # winograd data-transform row combos (Bt): padded rows 2ty+a
                for src, dst in ((myp, MYt), (mxp, MXt)):
                    nc.vector.tensor_sub(dst[:, :, 0], src[:, :, 0:14:2],
                                         src[:, :, 2:16:2])
                    nc.vector.tensor_add(dst[:, :, 1], src[:, :, 1:15:2],
                                         src[:, :, 2:16:2])
                    nc.vector.tensor_sub(dst[:, :, 2], src[:, :, 2:16:2],
                                         src[:, :, 1:15:2])
                    nc.vector.tensor_sub(dst[:, :, 3], src[:, :, 1:15:2],
                                         src[:, :, 3:16:2])
                # GW2[qc][u*4+v, ty, tx, n] = MY'[qc,u,ty,n] * MX'[qc,v,tx,n]
                # (per (u,v): ISA limits engine APs to 3 free dims);
                # qc=1/u>=2 runs on gpsimd to offload the DVE
                shg = [128, T7, T7, BATCH]
                for qc in range(2):
                    GW = GWb[qc][slot]
                    for u in range(4):
                        eng = nc.gpsimd if (qc == 1 and u >= 2) else nc.vector
                        myb = MYt[:, qc, u, :, None, :].to_broadcast(shg)
                        for v in range(4):
                            eng.tensor_tensor(
                                GW[:, 4 * u + v], myb,
                                MXt[:, qc, v, None, :, :].to_broadcast(shg),
                                ALU.mult)

            def box_math(n0, nn, eng):
                """fill per-axis index/weight arrays for boxes [n0, n0+nn)"""
                ns = slice(n0, n0 + nn)
                t, fr, ti = abx[:, 0:nn], bm_f4[:, 0:nn], bm_i4[:, 0:nn]
                eng.tensor_scalar_mul(t, bxb[:, ns], 0.125)
                eng.tensor_copy(ti, t)
                eng.tensor_copy(fr, ti)
                eng.tensor_tensor(ti.bitcast(F32), fr, t, ALU.is_gt)
                eng.tensor_sub(t, fr, ti.bitcast(F32))
                d, nlt, beq, adj = (bm_d[:, 0:nn], bm_n[:, 0:nn],
                                    bm_q[:, 0:nn], bm_h[:, 0:nn])
                for ax in range(2):  # 0: x (cols 0,2), 1: y (cols 1,3)
                    a_io, b_io = t[:, :, ax], t[:, :, 2 + ax]
                    eng.tensor_sub(d, b_io, a_io)
                    eng.tensor_scalar(nlt, d, 1.0, None, ALU.is_lt)
                    eng.tensor_scalar(beq, b_io, float(P), None, ALU.is_equal)
                    eng.tensor_mul(adj, nlt, beq)
                    eng.tensor_sub(a_io, a_io, adj)
                    eng.tensor_add(b_io, b_io, nlt)
                    eng.tensor_sub(b_io, b_io, adj)
                nwid, him1 = d, nlt
                s, frs, si, i0c = (bm_s[:, 0:nn], bm_f[:, 0:nn],
                                   bm_i[:, 0:nn], bm_a[:, 0:nn])
                sh3 = [128, nn, P]
                for ax, (I0, I1, Wf, OWf) in enumerate(
                    [(X0, X1, WX, OWX), (Y0, Y1, WY, OWY)]
                ):
                    ssl = (slice(None), ns)
                    lo_b = t[:, :, ax][:, :, None].to_broadcast(sh3)
                    eng.tensor_sub(nwid, t[:, :, 2 + ax], t[:, :, ax])
                    eng.tensor_scalar_sub(him1, nwid, 1.0)
                    h_b = him1[:, :, None].to_broadcast(sh3)
                    eng.tensor_tensor(s, nwid[:, :, None].to_broadcast(sh3),
                                      jc_b[:, None, :].to_broadcast(sh3), ALU.mult)
                    eng.tensor_scalar(s, s, RC14, -0.5, ALU.mult, ALU.add)
                    eng.tensor_scalar(s, s, 0.0, None, ALU.max)
                    eng.tensor_copy(si, s)
                    eng.tensor_copy(frs, si)
                    eng.tensor_tensor(si.bitcast(F32), frs, s, ALU.is_gt)
                    eng.tensor_sub(i0c, frs, si.bitcast(F32))
                    eng.tensor_tensor(i0c, i0c, h_b, ALU.min)
                    eng.tensor_sub(Wf[ssl], s, i0c)
                    eng.tensor_scalar(OWf[ssl], Wf[ssl], -1.0, 1.0, ALU.mult, ALU.add)
                    eng.tensor_add(I0[ssl], i0c, lo_b)
                    eng.tensor_scalar_add(i0c, i0c, 1.0)
                    eng.tensor_tensor(i0c, i0c, h_b, ALU.min)
                    eng.tensor_add(I1[ssl], i0c, lo_b)

            # ---------- phase 0 ----------
            with tc.tile_pool(name="ph0", bufs=1) as p0, \
                 tc.tile_pool(name="ps0", bufs=1, space="PSUM") as ps0, \
                 tc.tile_pool(name="psu", bufs=4, space="PSUM") as psu0:

                # --- tiny gating DMAs first (bx1 gates the broadcasts)
                ones1 = p0.tile([1, 128], F32, tag="ones1")
                nc.gpsimd.memset(ones1[:], 1.0)
                bx1 = p0.tile([1, NB * 4], F32, tag="bx1")
                nc.sync.dma_start(bx1[:], boxes.rearrange("n f -> (n f)")[None, :])
                cf1 = p0.tile([1, 80], F32, tag="cf1")
                nc.sync.dma_start(cf1[:], cf_d[:])
                nc.sync.dma_start(cpt[:], cp_d[:])

                # --- host pre-sliced feature rows/cols (28x28 of 96x128):
                #     gates the cf -> UC chain; 4 descriptors across queues
                R01c = p0.tile([128, 2, P, 2, P, 2], F32, tag="R01c")
                r01_f = R01c[:].rearrange("p c i r j s -> p (c i r j s)")
                QS = P * 2 * P * 2 // 2  # 392: half of one ch chunk
                for d in range(4):
                    nc.sync.dma_start(r01_f[:, d * QS : (d + 1) * QS],
                                      featsl[:, d * QS : (d + 1) * QS])

                # --- weights: crop half (gates UC2) before mask half
                Wc = p0.tile([128, 2, UV, 256], BF16, tag="Wc")
                nc.sync.dma_start(Wc[:].rearrange("p a b c -> p (a b c)"),
                                  wt_d[:, 2:4].rearrange("p a b c -> p (a b c)"))
                nc.sync.dma_start(Wm[:].rearrange("p a b c -> p (a b c)"),
                                  wt_d[:, 0:2].rearrange("p a b c -> p (a b c)"))
                # --- transformed mask batch 0
                nc.sync.dma_start(
                    XWb[0][:].rearrange("p c w t s n -> p (c w t s n)"),
                    xwm_d[:, 0:XW2SZ])

                # --- broadcasts via K=1 matmul with ones (PE is idle here)
                psb = ps0.tile([128, 256], F32, tag="psb")
                nc.tensor.matmul(psb[:], ones1[:], bx1[:])
                nc.scalar.copy(bxb[:].rearrange("p n f -> p (n f)"), psb[:])
                psf = ps0.tile([128, 80], F32, tag="psf")
                nc.tensor.matmul(psf[:], ones1[:], cf1[:])
                nc.scalar.copy(cft[:], psf[:])

                # --- one-time zeroing: UC[1] pad partitions (68..127) so qc1
                # matmuls run K=128; myp/mxp pad rows 0/15
                nc.gpsimd.memset(UC[1][:], 0.0)
                nc.gpsimd.memset(myp[:, :, 0, :], 0.0)
                nc.gpsimd.memset(myp[:, :, 15, :], 0.0)
                nc.gpsimd.memset(mxp[:, :, 0, :], 0.0)
                nc.gpsimd.memset(mxp[:, :, 15, :], 0.0)

                # --- concat-features (cf) first on DVE (gates UC)
                cfx = p0.tile([128, 2, P, 2, P], F32, tag="cfx")  # (ch, i, r, j)
                tmpx = p0.tile([128, 2, P, 2, P], F32, tag="tmpx")
                cfx_v = cfx[:].rearrange("p c i r j -> p c (i r) j")
                tmpx_v = tmpx[:].rearrange("p c i r j -> p c (i r) j")
                R01_m = R01c[:].rearrange("p c i r j s -> p c (i r) (j s)")
                shL = [128, 2, 2 * P, P]
                w1b = cft[:, None, None, 62:76].to_broadcast(shL)
                wb = cft[:, None, None, 48:62].to_broadcast(shL)
                nc.vector.tensor_tensor(cfx_v[:], R01_m[:, :, :, 0::2], w1b, ALU.mult)
                nc.vector.tensor_tensor(tmpx_v[:], R01_m[:, :, :, 1::2], wb, ALU.mult)
                nc.vector.tensor_add(cfx_v[:], cfx_v[:], tmpx_v[:])
                tmpy = p0.tile([128, 2, P, P], F32, tag="tmpy")
                shc = [128, 2, P, P]
                nc.vector.tensor_tensor(cfv[:], cfx[:, :, :, 0, :],
                                        cft[:, None, 28:42, None].to_broadcast(shc), ALU.mult)
                nc.vector.tensor_tensor(tmpy[:], cfx[:, :, :, 1, :],
                                        cft[:, None, 14:28, None].to_broadcast(shc), ALU.mult)
                nc.vector.tensor_add(cfv[:], cfv[:], tmpy[:])
                nc.vector.tensor_copy(cfb[:], cfv[:])

                # --- batch-0 operands
                box_math(0, BATCH, nc.vector)
                g_build(0)

                # --- UC2 build on PE: UC[qc][q, uv, o] =
                #     sum_ci cfb[ci, q] * Wc[ci_hi][ci, uv, o]; uv pairs so a
                #     512-f32 psum bank holds each output chunk
                cfv_f = cfb[:].rearrange("p c i j -> p c (i j)")
                for qc in range(2):
                    qn = 128 if qc == 0 else Q1
                    qs = slice(qc * 128, qc * 128 + qn)
                    for pr in range(8):
                        pt = psu0.tile([128, 2, 256], F32, tag="ups",
                                       name=f"ups{qc}_{pr}")
                        for cc in range(2):
                            nc.tensor.matmul(pt[:qn], cfv_f[:, cc, qs],
                                             Wc[:, cc, 2 * pr : 2 * pr + 2, :],
                                             start=(cc == 0), stop=(cc == 1))
                        nc.scalar.copy(UC[qc][:qn, 2 * pr : 2 * pr + 2, :], pt[:qn])

                # --- epilogue scalars
                nc.sync.dma_start(epi[:].rearrange("p a b -> p (a b)"),
                                  epi_d.rearrange("p a b -> p (a b)"))
                tmp_e = p0.tile([128, 2], F32, tag="tmp_e")
                eps_t = p0.tile([128, 1], F32, tag="eps_t")
                nc.vector.memset(eps_t[:], 1e-5)
                nc.scalar.activation(tmp_e[:], epi[:, 4, :], AF.Sqrt, bias=eps_t[:], scale=1.0)
                nc.vector.reciprocal(scale_e[:], tmp_e[:])
                nc.vector.tensor_mul(scale_e[:], scale_e[:], epi[:, 1, :])
                nc.vector.tensor_sub(bias_e[:], epi[:, 0, :], epi[:, 3, :])
                nc.vector.tensor_mul(bias_e[:], bias_e[:], scale_e[:])
                nc.vector.tensor_add(bias_e[:], bias_e[:], epi[:, 2, :])

            # ---------- main loop ----------
            with tc.tile_pool(name="loop", bufs=1) as lp, \
                 tc.tile_pool(name="gpool", bufs=2) as gp, \
                 tc.tile_pool(name="psv", bufs=2, space="PSUM") as psv:

                CH2 = [("m", 0), ("m", 1), ("c", 0), ("c", 1)]

                for b in range(NBATCH):
                    n0 = b * BATCH
                    slot = b % 2
                    XW = XWb[slot]
                    GWs = [GWb[0][slot], GWb[1][slot]]
                    last = b == NBATCH - 1

                    for oc in range(2):
                        ost = lp.tile([128, BATCH, P, P], F32, tag="ost",
                                      name=f"ost{b}_{oc}")
                        # Mc: evacuated winograd products [uv, (ty tx), n]
                        Mc = lp.tile([128, UV, NT, BATCH], BF16, tag="Mc",
                                     name=f"Mc_{b}_{oc}")
                        Tt = gp.tile([128, 2, 4, NT, BATCH], BF16, tag="Tt",
                                     name=f"T_{b}_{oc}")
                        tT = gp.tile([128, 4, NT, BATCH], BF16, tag="tT",
                                     name=f"tT_{b}_{oc}")
                        tY = gp.tile([128, BATCH, NT], BF16, tag="tY",
                                     name=f"tY_{b}_{oc}")
                        Yp = gp.tile([128, BATCH, P, P], BF16, tag="Yp",
                                     name=f"Yp_{b}_{oc}")

                        # 4 psum groups of 4 banks; one grouped evacuation per
                        # 4 uv positions keeps scalar-engine op count low
                        for g in range(4):
                            PG = psv.tile([128, 4, 512], F32, tag="PG",
                                          name=f"PG_{b}_{oc}_{g}")
                            # all mask-half chunks of the group first, then
                            # the crop half: per-box GW2 frames get an extra
                            # ~2.6us of pipeline slack each group
                            for kind, c in CH2:
                                for j in range(4):
                                    uv = 4 * g + j
                                    out = PG[:, j, 0:NT * BATCH].rearrange(
                                        "p (t n) -> p t n", n=BATCH)
                                    if kind == "m":
                                        lhsT = Wm[:, c, uv, oc * 128 : oc * 128 + 128]
                                        rhs = XW[:, c, uv]
                                    else:
                                        lhsT = UC[c][:, uv, oc * 128 : oc * 128 + 128]
                                        rhs = GWs[c][:, uv]
                                    nc.tensor.matmul(
                                        out, lhsT, rhs,
                                        start=(kind == "m" and c == 0),
                                        stop=(kind == "c" and c == 1))
                            nc.scalar.copy(
                                Mc[:, 4 * g : 4 * g + 4].rearrange(
                                    "p w t n -> p w (t n)"),
                                PG[:, :, 0:NT * BATCH])
                        # inverse transform, x then y (A^T M A):
                        # v-combine on DVE as 4 big strided ops over all u
                        nc.vector.tensor_add(tT[:], Mc[:, 0::4], Mc[:, 1::4])
                        nc.vector.tensor_add(Tt[:, 0], tT[:], Mc[:, 2::4])
                        nc.vector.tensor_sub(tT[:], Mc[:, 1::4], Mc[:, 2::4])
                        nc.vector.tensor_sub(Tt[:, 1], tT[:], Mc[:, 3::4])
                        # y-combine on gpsimd, writing the 2x2-quadrant
                        # interleave of Yp directly ([n, ty, tx] reads)
                        Ypq = [[Yp[:, :, ey::2, ex::2] for ex in range(2)]
                               for ey in range(2)]

                        def Ts(ex, u):
                            return Tt[:, ex, u].transpose([0, 2, 1]).rearrange(
                                "p n (t s) -> p n t s", s=T7)

                        tYv = tY[:].rearrange("p n (t s) -> p n t s", s=T7)
                        for ex in range(2):
                            nc.gpsimd.tensor_add(tYv[:], Ts(ex, 0), Ts(ex, 1))
                            nc.gpsimd.tensor_add(Ypq[0][ex], tYv[:], Ts(ex, 2))
                            nc.gpsimd.tensor_sub(tYv[:], Ts(ex, 1), Ts(ex, 2))
                            nc.gpsimd.tensor_sub(Ypq[1][ex], tYv[:], Ts(ex, 3))
                        # BN + ReLU in one contiguous activation, then store
                        Ypf = Yp[:].rearrange("p n i j -> p n (i j)")
                        ost_f = ost[:].rearrange("p n i j -> p n (i j)")
                        nsplit = 4 if last and oc == 1 else 2
                        nb_s = BATCH // nsplit
                        for h in range(nsplit):
                            ns = slice(h * nb_s, h * nb_s + nb_s)
                            nc.scalar.activation(
                                ost_f[:, ns], Ypf[:, ns], AF.Relu,
                                bias=bias_e[:, oc : oc + 1],
                                scale=scale_e[:, oc : oc + 1])
                            nc.sync.dma_start(
                                out_v[:, oc, n0 + h * nb_s : n0 + (h + 1) * nb_s],
                                ost_f[:, ns])
                        # next batch's operands, emitted mid-batch so DVE/Pool
                        # produce them while the PE runs this batch's passes
                        if oc == 0 and b + 1 < NBATCH:
                            if b in (0, 2):
                                box_math(8 + (b // 2) * BW, BW, nc.vector)
                            nc.sync.dma_start(
                                XWb[(b + 1) % 2][:].rearrange(
                                    "p c w t s n -> p (c w t s n)"),
                                xwm_d[:, (b + 1) * XW2SZ : (b + 2) * XW2SZ])
                            g_build(b + 1)

    nc.compile()
    return nc


# ---------------------------------------------------------------------------
# host-side sharding / unsharding
# ---------------------------------------------------------------------------

def _xwm_host(mask):
    """2-D winograd transform of mask features on host: [NB,256,14,14] ->
    [128 cp, NBATCH, 2 ci, 16 uv, 7 ty, 7 tx, 8 n] bf16, flattened."""
    import ml_dtypes
    v = mask.reshape(NB, 2, 128, P, P)
    vp = np.zeros((NB, 2, 128, 16, 16), np.float32)
    vp[:, :, :, 1:15, 1:15] = v
    sw = np.lib.stride_tricks.sliding_window_view(vp, (4, 4), axis=(3, 4))
    til = sw[:, :, :, ::2, ::2]                    # [n, ci, cp, ty, tx, a, b]
    V = np.einsum('ua,vb,ncpyxab->pncuvyx', BT, BT, til,
                  optimize=True)                   # [cp, n, ci, u, v, ty, tx]
    V = V.reshape(128, NBATCH, BATCH, 2, UV, T7, T7)
    V = V.transpose(0, 1, 3, 4, 5, 6, 2)           # [cp, b, ci, uv, ty, tx, j]
    return np.ascontiguousarray(V.reshape(128, -1)).astype(ml_dtypes.bfloat16)


def _prep_in_maps(features, proposal_boxes, mask_features, conv_w, conv_b,
                  bn_gamma, bn_beta, bn_mean, bn_var):
    features = np.asarray(features, dtype=np.float32)
    proposal_boxes = np.asarray(proposal_boxes, dtype=np.float32)
    mask_features = np.asarray(mask_features, dtype=np.float32)
    conv_w = np.asarray(conv_w, dtype=np.float32)
    import ml_dtypes
    # weight layout: 2-D winograd transform U = G g G^T per (cin, cout):
    # [cout=256, cin=512, 3, 3] -> [cin_par=128, cin_hi=4, uv(16), cout], bf16
    wf = conv_w.reshape(256, 4, 128, 3, 3)                 # [o, hi, par, dy, dx]
    ww = np.einsum('uy,vx,ohpyx->phuvo', GM, GM, wf)       # [par, hi, u, v, o]
    wt = np.ascontiguousarray(ww.reshape(128, 4, UV, 256)).astype(ml_dtypes.bfloat16)
    epi = np.stack([np.asarray(x, dtype=np.float32) for x in
                    (conv_b, bn_gamma, bn_beta, bn_mean, bn_var)])  # [5, 256]
    epi = np.ascontiguousarray(epi.reshape(5, 2, 128).transpose(2, 0, 1)).astype(np.float32)
    cp = _consts_p()
    cfc = _consts_f()

    in_maps = []
    for i in range(N_CORES):
        img = i // (N_CORES // 2)
        n0 = (i * NB) % 256
        fimg = features[img]
        fsl = fimg[:, np.stack([YS0, YS1], axis=1), :]          # [256, 14, 2, W]
        fsl = fsl[:, :, :, np.stack([XS0, XS1], axis=1)]        # [256, 14, 2, 14, 2]
        fsl = np.ascontiguousarray(
            fsl.reshape(2, 128, P * 2 * P * 2).transpose(1, 0, 2).reshape(128, -1))
        in_maps.append({
            "featsl": fsl,
            "boxes": np.ascontiguousarray(proposal_boxes[img, n0 : n0 + NB]),
            "xwm": _xwm_host(mask_features[i * NB : (i + 1) * NB]),
            "wt": wt,
            "epi": epi,
            "consts_p": cp,
            "consts_f": cfc,
        })
    return in_maps


_NC_CACHE = {}


def _get_nc():
    if "nc" not in _NC_CACHE:
        _NC_CACHE["nc"] = build_kernel()
    return _NC_CACHE["nc"]


def _install_ntff_shim():
    """antenv.axon_hooks is missing in this image; shim it so trace=True works."""
    try:
        import antenv
        if hasattr(antenv, "axon_hooks"):
            return
        from trn_agent_boot.trn_boot import _ntff_profile_via_ctypes
        mod = types.ModuleType("antenv.axon_hooks")
        _h = [None]
        mod.set_axon_ntff_profile_hook = lambda h: _h.__setitem__(0, h)
        mod.get_axon_ntff_profile_hook = lambda: _h[0]
        sys.modules["antenv.axon_hooks"] = mod
        antenv.axon_hooks = mod
        mod.set_axon_ntff_profile_hook(_ntff_profile_via_ctypes("/opt/axon/libaxon_pjrt.so"))
    except Exception:
        pass


def run(trace=False, tmpdir=None, **inputs):
    from concourse.bass_utils import run_bass_kernel_spmd

    if trace:
        _install_ntff_shim()
    nc = _get_nc()
    in_maps = _prep_in_maps(**inputs)
    res = run_bass_kernel_spmd(nc, in_maps, core_ids=list(range(N_CORES)),
                               trace=trace, tmpdir=tmpdir)
    out = np.concatenate([np.asarray(res.results[i]["out"]) for i in range(N_CORES)], axis=0)
    return out.astype(np.float32), res


def kernel(**inputs):
    out, _ = run(trace=False, **inputs)
    return out


# revision 55
# speedup vs baseline: 1.0374x; 1.0374x over previous
"""Trainium2 Bass kernel for nn_AddMaskHead (ROI mask head: bilinear pool +
concat + conv3x3 + BN + ReLU).

Self-contained: hardcodes shapes B=2, N=256 (512 boxes), C=256, H=96, W=128,
P=14. Shards data-parallel over the 512 boxes across 8 NeuronCores (64
boxes/core; each core's boxes all come from a single image).

Conv strategy: 2-D Winograd F(2x2, 3x3). Each 14x14 output splits into 7x7
tiles of 2x2; per tile the conv needs 16 (u,v) multiply positions instead of
36 (4 muls/output vs 9 direct, vs 6 for the previous 1-D scheme). The mask
half of the concat input is Winograd-transformed on the host (static linear
map, shipped as bf16 -> same DMA bytes as the raw f32 mask). The ROI-pooled
half is folded through per-box separable interpolation matrices: UC2 =
(upsampled concat feats x transformed weights) is box-independent and built
once on the PE; per box only the rank-1 frame GW2 = MY' (x) MX' is built on
the DVE (bf16, n-innermost so every op hits the packed 2-byte fast path).
Matmuls accumulate 4 K=128 chunks into one 392-column PSUM bank per (u,v)
position (8 boxes x 49 tiles), uv-major so banks recycle through a
scalar/gpsimd evacuation pipeline while the PE streams on.
"""

import sys, os, types

sys.path.insert(0, "/opt/trn_rl_repo")

import numpy as np
import concourse.bass as bass
import concourse.mybir as mybir
import concourse.tile as tile
from concourse import bacc
from concourse.masks import make_identity

F32 = mybir.dt.float32
BF16 = mybir.dt.bfloat16
I32 = mybir.dt.int32
ALU = mybir.AluOpType
AF = mybir.ActivationFunctionType

N_CORES = 8
NB = 64            # boxes per core
BATCH = 8          # boxes per inner batch
NBATCH = NB // BATCH
P = 14             # pooler resolution
C = 256            # channels
H, W = 96, 128     # feature map
PQ = P * P         # 196
Q1 = PQ - 128      # 68 (q-chunk 1 size)
T7 = 7             # winograd tiles per axis
NT = T7 * T7       # 49 tiles
UV = 16            # winograd multiply positions per tile
XW2SZ = 2 * UV * NT * BATCH   # per-batch transformed-mask elems per partition


def _axis_static(in_s, out_s=P):
    # mirrors reference._resize_bilinear axis() in exact f32 arithmetic
    s = (np.arange(out_s, dtype=np.float32) + np.float32(0.5)) * np.float32(in_s / out_s) - np.float32(0.5)
    s = np.maximum(s, np.float32(0.0))
    i0 = np.minimum(np.floor(s).astype(np.int32), in_s - 1)
    i1 = np.minimum(i0 + 1, in_s - 1)
    w = (s - i0.astype(np.float32)).astype(np.float32)
    return i0, i1, w


YS0, YS1, WYS = _axis_static(H)
XS0, XS1, WXS = _axis_static(W)

BT = np.array([[1, 0, -1, 0], [0, 1, 1, 0], [0, -1, 1, 0], [0, 1, 0, -1]],
              np.float32)
GM = np.array([[1, 0, 0], [.5, .5, .5], [.5, -.5, .5], [0, 0, 1]], np.float32)


def _consts_p():
    # per-partition constants: [128, 4] = (yv_q0, xv_q0, yv_q1, xv_q1); -1 pads
    arr = np.full((128, 4), -1.0, dtype=np.float32)
    for p in range(128):
        arr[p, 0] = (p // P)
        arr[p, 1] = (p % P)
    for p in range(Q1):
        q = 128 + p
        arr[p, 2] = (q // P)
        arr[p, 3] = (q % P)
    return arr


def _consts_f():
    # free-dim constants (broadcast to all partitions on device):
    # [0:14] jc = arange(14)+0.5; [14:28] wys; [28:42] 1-wys;
    # [48:62] wxs; [62:76] 1-wxs
    arr = np.zeros((1, 80), dtype=np.float32)
    arr[0, 0:14] = np.arange(P, dtype=np.float32) + np.float32(0.5)
    arr[0, 14:28] = WYS
    arr[0, 28:42] = np.float32(1.0) - WYS
    arr[0, 48:62] = WXS
    arr[0, 62:76] = np.float32(1.0) - WXS
    return arr


def build_kernel():
    nc = bacc.Bacc(None)

    featsl = nc.declare_dram_parameter("featsl", [128, 2 * P * 2 * P * 2], F32,
                                       isOutput=False)
    boxes = nc.declare_dram_parameter("boxes", [NB, 4], F32, isOutput=False)
    # host 2-D-winograd mask features [cp, batch, ci, uv, ty, tx, n]
    xwm_d = nc.declare_dram_parameter("xwm", [128, NBATCH * XW2SZ], BF16,
                                      isOutput=False)
    wt_d = nc.declare_dram_parameter("wt", [128, 4, UV, 256], BF16, isOutput=False)
    epi_d = nc.declare_dram_parameter("epi", [128, 5, 2], F32, isOutput=False)
    cp_d = nc.declare_dram_parameter("consts_p", [128, 4], F32, isOutput=False)
    cf_d = nc.declare_dram_parameter("consts_f", [1, 80], F32, isOutput=False)
    out_d = nc.declare_dram_parameter("out", [NB, C, P, P], F32, isOutput=True)

    out_v = out_d.rearrange("n (oh op) i j -> op oh n (i j)", op=128)

    RC14 = float(np.float32(1.0) / np.float32(P))

    with tile.TileContext(nc) as tc:
        with tc.tile_pool(name="persist", bufs=1) as pp:
            # ---------- persistent tiles ----------
            # mask-half transformed weights [ci_par, ci_hi, uv, o]
            Wm = pp.tile([128, 2, UV, 256], BF16, tag="Wm")
            # UC2[qc][q, uv, o] = sum_ci cf[ci, q] * Wc2[uv][ci, o]
            UC = [pp.tile([128, UV, 256], BF16, tag=f"UC{qc}", name=f"UC{qc}")
                  for qc in range(2)]
            # host-transformed mask features per batch [cp, ci, uv, ty, tx, n]
            XWb = [pp.tile([128, 2, UV, T7, T7, BATCH], BF16, tag=f"xw{i}",
                           name=f"xw{i}") for i in range(2)]
            # per-box winograd interp frames per q-chunk (double-buffered)
            GWb = [[pp.tile([128, UV, T7, T7, BATCH], BF16, tag=f"gw{qc}{i}",
                            name=f"gw{qc}{i}") for i in range(2)]
                   for qc in range(2)]
            cpt = pp.tile([128, 4], F32, tag="cpt")
            cpb = pp.tile([128, 4], BF16, tag="cpb")
            cft = pp.tile([128, 80], F32, tag="cft")
            epi = pp.tile([128, 5, 2], F32, tag="epi")
            scale_e = pp.tile([128, 2], F32, tag="scale_e")
            bias_e = pp.tile([128, 2], F32, tag="bias_e")
            # per-box interpolation data for all 64 boxes (bf16: indices are
            # small ints = exact; lerp weights lose ~2^-9 = fine at 2e-2)
            BW = 32  # box_math batch width (boxes per call after batch 0)
            Y0 = pp.tile([128, NB, P], BF16, tag="Y0")
            Y1 = pp.tile([128, NB, P], BF16, tag="Y1")
            WY = pp.tile([128, NB, P], BF16, tag="WY")
            OWY = pp.tile([128, NB, P], BF16, tag="OWY")
            X0 = pp.tile([128, NB, P], BF16, tag="X0")
            X1 = pp.tile([128, NB, P], BF16, tag="X1")
            WX = pp.tile([128, NB, P], BF16, tag="WX")
            OWX = pp.tile([128, NB, P], BF16, tag="OWX")
            # padded interp row/col weights [q, qc, 16, n] (rows 0/15 zero),
            # built in 32-box halves to amortize DVE op overhead
            MW = 32
            myp = pp.tile([128, 2, 16, MW], BF16, tag="myp")
            mxp = pp.tile([128, 2, 16, MW], BF16, tag="mxp")
            MYt = pp.tile([128, 2, 4, T7, MW], BF16, tag="MYt")
            MXt = pp.tile([128, 2, 4, T7, MW], BF16, tag="MXt")
            # box-math temps (BW-wide; the 8-box phase-0 call uses a slice)
            bxb = pp.tile([128, NB, 4], F32, tag="bxb")
            abx = pp.tile([128, BW, 4], F32, tag="abx")
            bm_i4 = pp.tile([128, BW, 4], I32, tag="bm_i4")
            bm_f4 = pp.tile([128, BW, 4], F32, tag="bm_f4")
            bm_s = pp.tile([128, BW, P], F32, tag="bm_s")
            bm_f = pp.tile([128, BW, P], F32, tag="bm_f")
            bm_i = pp.tile([128, BW, P], I32, tag="bm_i")
            bm_a = pp.tile([128, BW, P], F32, tag="bm_a")
            bm_d = pp.tile([128, BW], F32, tag="bm_d")
            bm_n = pp.tile([128, BW], F32, tag="bm_n")
            bm_q = pp.tile([128, BW], F32, tag="bm_q")
            bm_h = pp.tile([128, BW], F32, tag="bm_h")
            cfv = pp.tile([128, 2, P, P], F32, tag="cfv")
            cfb = pp.tile([128, 2, P, P], BF16, tag="cfb")
            cmpb = pp.tile([128, 2, P, MW], BF16, tag="cmpb")

            jc_b = cft[:, 0:14]

            def mm_build(n0, nn):
                """my/mx interp masks + winograd row combos MY'/MX' for box
                rows [n0, n0+nn), written at columns [n0 % MW, ...) of the
                32-box ping-pong arrays (DVE; batched)."""
                ns = slice(n0, n0 + nn)
                c0 = n0 % MW
                cs = slice(c0, c0 + nn)
                sh = [128, 2, P, nn]
                yv = cpb[:, 0:4:2, None, None].to_broadcast(sh)
                xv = cpb[:, 1:4:2, None, None].to_broadcast(sh)

                def tr(a):
                    # transposed read of a box array: [r, n] from [n, r]
                    return a[:, ns].transpose([0, 2, 1])[:, None].to_broadcast(sh)

                my = myp[:, :, 1:15, cs]
                mx = mxp[:, :, 1:15, cs]
                cmp = cmpb[:, :, :, 0:nn]
                nc.vector.tensor_tensor(my, tr(Y0), yv, ALU.is_equal)
                nc.vector.tensor_mul(my, my, tr(OWY))
                nc.vector.tensor_tensor(cmp, tr(Y1), yv, ALU.is_equal)
                nc.vector.tensor_mul(cmp, cmp, tr(WY))
                nc.vector.tensor_add(my, my, cmp)
                nc.vector.tensor_tensor(mx, tr(X0), xv, ALU.is_equal)
                nc.vector.tensor_mul(mx, mx, tr(OWX))
                nc.vector.tensor_tensor(cmp, tr(X1), xv, ALU.is_equal)
                nc.vector.tensor_mul(cmp, cmp, tr(WX))
                nc.vector.tensor_add(mx, mx, cmp)
                # winograd data-transform row combos (Bt): padded rows 2ty+a
                for src, dst in ((myp, MYt), (mxp, MXt)):
                    nc.vector.tensor_sub(dst[:, :, 0, :, cs], src[:, :, 0:14:2, cs],
                                         src[:, :, 2:16:2, cs])
                    nc.vector.tensor_add(dst[:, :, 1, :, cs], src[:, :, 1:15:2, cs],
                                         src[:, :, 2:16:2, cs])
                    nc.vector.tensor_sub(dst[:, :, 2, :, cs], src[:, :, 2:16:2, cs],
                                         src[:, :, 1:15:2, cs])
                    nc.vector.tensor_sub(dst[:, :, 3, :, cs], src[:, :, 1:15:2, cs],
                                         src[:, :, 3:16:2, cs])

            def g_build(b):
                """GW2[qc][u*4+v, ty, tx, n] = MY'[qc,u,ty,n] * MX'[qc,v,tx,n]
                for batch b; 32 rank-1 ops split half DVE / half gpsimd."""
                slot = b % 2
                c0 = (b * BATCH) % MW
                ns = slice(c0, c0 + BATCH)
                shg = [128, T7, T7, BATCH]
                for qc in range(2):
                    GW = GWb[qc][slot]
                    for u in range(4):
                        eng = nc.gpsimd if u >= 2 else nc.vector
                        myb = MYt[:, qc, u, :, None, ns].to_broadcast(shg)
                        for v in range(4):
                            eng.tensor_tensor(
                                GW[:, 4 * u + v], myb,
                                MXt[:, qc, v, None, :, ns].to_broadcast(shg),
                                ALU.mult)

            def box_math(n0, nn, eng):
                """fill per-axis index/weight arrays for boxes [n0, n0+nn)"""
                ns = slice(n0, n0 + nn)
                # floor: the f32->i32 copy rounds to nearest, so subtract
                # the (rounded > original) mask to get floor
                t, fr, ti = abx[:, 0:nn], bm_f4[:, 0:nn], bm_i4[:, 0:nn]
                eng.tensor_scalar_mul(t, bxb[:, ns], 0.125)
                eng.tensor_copy(ti, t)
                eng.tensor_copy(fr, ti)
                eng.tensor_tensor(ti.bitcast(F32), fr, t, ALU.is_gt)
                eng.tensor_sub(t, fr, ti.bitcast(F32))
                d, nlt, beq, adj = (bm_d[:, 0:nn], bm_n[:, 0:nn],
                                    bm_q[:, 0:nn], bm_h[:, 0:nn])
                for ax in range(2):  # 0: x (cols 0,2), 1: y (cols 1,3)
                    a_io, b_io = t[:, :, ax], t[:, :, 2 + ax]
                    eng.tensor_sub(d, b_io, a_io)
                    eng.tensor_scalar(nlt, d, 1.0, None, ALU.is_lt)
                    eng.tensor_scalar(beq, b_io, float(P), None, ALU.is_equal)
                    eng.tensor_mul(adj, nlt, beq)
                    eng.tensor_sub(a_io, a_io, adj)
                    eng.tensor_add(b_io, b_io, nlt)
                    eng.tensor_sub(b_io, b_io, adj)
                nwid, him1 = d, nlt
                s, frs, si, i0c = (bm_s[:, 0:nn], bm_f[:, 0:nn],
                                   bm_i[:, 0:nn], bm_a[:, 0:nn])
                sh3 = [128, nn, P]
                for ax, (I0, I1, Wf, OWf) in enumerate(
                    [(X0, X1, WX, OWX), (Y0, Y1, WY, OWY)]
                ):
                    ssl = (slice(None), ns)
                    lo_b = t[:, :, ax][:, :, None].to_broadcast(sh3)
                    eng.tensor_sub(nwid, t[:, :, 2 + ax], t[:, :, ax])
                    eng.tensor_scalar_sub(him1, nwid, 1.0)
                    h_b = him1[:, :, None].to_broadcast(sh3)
                    eng.tensor_tensor(s, nwid[:, :, None].to_broadcast(sh3),
                                      jc_b[:, None, :].to_broadcast(sh3), ALU.mult)
                    eng.tensor_scalar(s, s, RC14, -0.5, ALU.mult, ALU.add)
                    eng.tensor_scalar(s, s, 0.0, None, ALU.max)
                    eng.tensor_copy(si, s)
                    eng.tensor_copy(frs, si)
                    eng.tensor_tensor(si.bitcast(F32), frs, s, ALU.is_gt)
                    eng.tensor_sub(i0c, frs, si.bitcast(F32))
                    eng.tensor_tensor(i0c, i0c, h_b, ALU.min)
                    eng.tensor_sub(Wf[ssl], s, i0c)
                    eng.tensor_scalar(OWf[ssl], Wf[ssl], -1.0, 1.0, ALU.mult, ALU.add)
                    eng.tensor_add(I0[ssl], i0c, lo_b)
                    eng.tensor_scalar_add(i0c, i0c, 1.0)
                    eng.tensor_tensor(i0c, i0c, h_b, ALU.min)
                    eng.tensor_add(I1[ssl], i0c, lo_b)

            # ---------- phase 0 ----------
            with tc.tile_pool(name="ph0", bufs=1) as p0, \
                 tc.tile_pool(name="ps0", bufs=1, space="PSUM") as ps0, \
                 tc.tile_pool(name="psu", bufs=4, space="PSUM") as psu0:

                # --- tiny gating DMAs first (bx1 gates the broadcasts)
                ones1 = p0.tile([1, 128], F32, tag="ones1")
                nc.gpsimd.memset(ones1[:], 1.0)
                bx1 = p0.tile([1, NB * 4], F32, tag="bx1")
                nc.sync.dma_start(bx1[:], boxes.rearrange("n f -> (n f)")[None, :])
                cf1 = p0.tile([1, 80], F32, tag="cf1")
                nc.sync.dma_start(cf1[:], cf_d[:])
                nc.sync.dma_start(cpt[:], cp_d[:])
                nc.vector.tensor_copy(cpb[:], cpt[:])

                # --- host pre-sliced feature rows/cols (28x28 of 96x128):
                #     gates the cf -> UC chain; 4 descriptors across queues
                R01c = p0.tile([128, 2, P, 2, P, 2], F32, tag="R01c")
                r01_f = R01c[:].rearrange("p c i r j s -> p (c i r j s)")
                QS = P * 2 * P * 2 // 2  # 392: half of one ch chunk
                for d in range(4):
                    nc.sync.dma_start(r01_f[:, d * QS : (d + 1) * QS],
                                      featsl[:, d * QS : (d + 1) * QS])

                # --- weights: crop half (gates UC2) before mask half
                Wc = p0.tile([128, 2, UV, 256], BF16, tag="Wc")
                nc.sync.dma_start(Wc[:].rearrange("p a b c -> p (a b c)"),
                                  wt_d[:, 2:4].rearrange("p a b c -> p (a b c)"))
                nc.sync.dma_start(Wm[:].rearrange("p a b c -> p (a b c)"),
                                  wt_d[:, 0:2].rearrange("p a b c -> p (a b c)"))
                # --- transformed mask batch 0
                nc.sync.dma_start(
                    XWb[0][:].rearrange("p c w t s n -> p (c w t s n)"),
                    xwm_d[:, 0:XW2SZ])

                # --- broadcasts via K=1 matmul with ones (PE is idle here)
                psb = ps0.tile([128, 256], F32, tag="psb")
                nc.tensor.matmul(psb[:], ones1[:], bx1[:])
                nc.scalar.copy(bxb[:].rearrange("p n f -> p (n f)"), psb[:])
                psf = ps0.tile([128, 80], F32, tag="psf")
                nc.tensor.matmul(psf[:], ones1[:], cf1[:])
                nc.scalar.copy(cft[:], psf[:])

                # --- one-time zeroing: UC[1] pad partitions (68..127) so qc1
                # matmuls run K=128; myp/mxp pad rows 0/15
                nc.gpsimd.memset(UC[1][:], 0.0)
                nc.gpsimd.memset(myp[:, :, 0, :], 0.0)
                nc.gpsimd.memset(myp[:, :, 15, :], 0.0)
                nc.gpsimd.memset(mxp[:, :, 0, :], 0.0)
                nc.gpsimd.memset(mxp[:, :, 15, :], 0.0)

                # --- concat-features (cf) first on DVE (gates UC)
                cfx = p0.tile([128, 2, P, 2, P], F32, tag="cfx")  # (ch, i, r, j)
                tmpx = p0.tile([128, 2, P, 2, P], F32, tag="tmpx")
                cfx_v = cfx[:].rearrange("p c i r j -> p c (i r) j")
                tmpx_v = tmpx[:].rearrange("p c i r j -> p c (i r) j")
                R01_m = R01c[:].rearrange("p c i r j s -> p c (i r) (j s)")
                shL = [128, 2, 2 * P, P]
                w1b = cft[:, None, None, 62:76].to_broadcast(shL)
                wb = cft[:, None, None, 48:62].to_broadcast(shL)
                nc.vector.tensor_tensor(cfx_v[:], R01_m[:, :, :, 0::2], w1b, ALU.mult)
                nc.vector.tensor_tensor(tmpx_v[:], R01_m[:, :, :, 1::2], wb, ALU.mult)
                nc.vector.tensor_add(cfx_v[:], cfx_v[:], tmpx_v[:])
                tmpy = p0.tile([128, 2, P, P], F32, tag="tmpy")
                shc = [128, 2, P, P]
                nc.vector.tensor_tensor(cfv[:], cfx[:, :, :, 0, :],
                                        cft[:, None, 28:42, None].to_broadcast(shc), ALU.mult)
                nc.vector.tensor_tensor(tmpy[:], cfx[:, :, :, 1, :],
                                        cft[:, None, 14:28, None].to_broadcast(shc), ALU.mult)
                nc.vector.tensor_add(cfv[:], cfv[:], tmpy[:])
                nc.vector.tensor_copy(cfb[:], cfv[:])

                # --- batch-0 operands
                box_math(0, BATCH, nc.vector)
                mm_build(0, BATCH)
                g_build(0)

                # --- UC2 build on PE: UC[qc][q, uv, o] =
                #     sum_ci cfb[ci, q] * Wc[ci_hi][ci, uv, o]; uv pairs so a
                #     512-f32 psum bank holds each output chunk
                cfv_f = cfb[:].rearrange("p c i j -> p c (i j)")
                for qc in range(2):
                    qn = 128 if qc == 0 else Q1
                    qs = slice(qc * 128, qc * 128 + qn)
                    for pr in range(8):
                        pt = psu0.tile([128, 2, 256], F32, tag="ups",
                                       name=f"ups{qc}_{pr}")
                        for cc in range(2):
                            nc.tensor.matmul(pt[:qn], cfv_f[:, cc, qs],
                                             Wc[:, cc, 2 * pr : 2 * pr + 2, :],
                                             start=(cc == 0), stop=(cc == 1))
                        nc.scalar.copy(UC[qc][:qn, 2 * pr : 2 * pr + 2, :], pt[:qn])

                # --- epilogue scalars
                nc.sync.dma_start(epi[:].rearrange("p a b -> p (a b)"),
                                  epi_d.rearrange("p a b -> p (a b)"))
                tmp_e = p0.tile([128, 2], F32, tag="tmp_e")
                eps_t = p0.tile([128, 1], F32, tag="eps_t")
                nc.vector.memset(eps_t[:], 1e-5)
                nc.scalar.activation(tmp_e[:], epi[:, 4, :], AF.Sqrt, bias=eps_t[:], scale=1.0)
                nc.vector.reciprocal(scale_e[:], tmp_e[:])
                nc.vector.tensor_mul(scale_e[:], scale_e[:], epi[:, 1, :])
                nc.vector.tensor_sub(bias_e[:], epi[:, 0, :], epi[:, 3, :])
                nc.vector.tensor_mul(bias_e[:], bias_e[:], scale_e[:])
                nc.vector.tensor_add(bias_e[:], bias_e[:], epi[:, 2, :])

            # ---------- main loop ----------
            with tc.tile_pool(name="loop", bufs=1) as lp, \
                 tc.tile_pool(name="gpool", bufs=1) as gp, \
                 tc.tile_pool(name="psv", bufs=2, space="PSUM") as psv:

                CH2 = [("m", 0), ("m", 1), ("c", 0), ("c", 1)]

                for b in range(NBATCH):
                    n0 = b * BATCH
                    slot = b % 2
                    XW = XWb[slot]
                    GWs = [GWb[0][slot], GWb[1][slot]]
                    last = b == NBATCH - 1

                    for oc in range(2):
                        ost = lp.tile([128, BATCH, P, P], F32, tag="ost",
                                      name=f"ost{b}_{oc}")
                        # Mc: evacuated winograd products [uv, (ty tx), n]
                        Mc = lp.tile([128, UV, NT, BATCH], BF16, tag="Mc",
                                     name=f"Mc_{b}_{oc}")
                        Tt = gp.tile([128, 2, 4, NT, BATCH], BF16, tag="Tt",
                                     name=f"T_{b}_{oc}")
                        tT = gp.tile([128, 4, NT, BATCH], BF16, tag="tT",
                                     name=f"tT_{b}_{oc}")
                        tY = gp.tile([128, 2, NT, BATCH], BF16, tag="tY",
                                     name=f"tY_{b}_{oc}")
                        Yp = gp.tile([128, 2, 2, NT, BATCH], BF16, tag="Yp",
                                     name=f"Yp_{b}_{oc}")

                        # 4 psum groups of 4 banks; one grouped evacuation per
                        # 4 uv positions keeps scalar-engine op count low
                        for g in range(4):
                            PG = psv.tile([128, 4, 512], F32, tag="PG",
                                          name=f"PG_{b}_{oc}_{g}")
                            # all mask-half chunks of the group first, then
                            # the crop half: per-box GW2 frames get an extra
                            # ~2.6us of pipeline slack each group
                            for kind, c in CH2:
                                for j in range(4):
                                    uv = 4 * g + j
                                    out = PG[:, j, 0:NT * BATCH].rearrange(
                                        "p (t n) -> p t n", n=BATCH)
                                    if kind == "m":
                                        lhsT = Wm[:, c, uv, oc * 128 : oc * 128 + 128]
                                        rhs = XW[:, c, uv]
                                    else:
                                        lhsT = UC[c][:, uv, oc * 128 : oc * 128 + 128]
                                        rhs = GWs[c][:, uv]
                                    nc.tensor.matmul(
                                        out, lhsT, rhs,
                                        start=(kind == "m" and c == 0),
                                        stop=(kind == "c" and c == 1))
                            nc.scalar.copy(
                                Mc[:, 4 * g : 4 * g + 4].rearrange(
                                    "p w t n -> p w (t n)"),
                                PG[:, :, 0:NT * BATCH])
                        # inverse transform, x then y (A^T M A):
                        # v-combine on DVE as 4 big strided ops over all u
                        nc.vector.tensor_add(tT[:], Mc[:, 0::4], Mc[:, 1::4])
                        nc.vector.tensor_add(Tt[:, 0], tT[:], Mc[:, 2::4])
                        nc.vector.tensor_sub(tT[:], Mc[:, 1::4], Mc[:, 2::4])
                        nc.vector.tensor_sub(Tt[:, 1], tT[:], Mc[:, 3::4])
                        # y-combine on gpsimd: packed [2ex, (ty tx), n] ops
                        nc.gpsimd.tensor_add(tY[:], Tt[:, :, 0], Tt[:, :, 1])
                        nc.gpsimd.tensor_add(Yp[:, 0], tY[:], Tt[:, :, 2])
                        nc.gpsimd.tensor_sub(tY[:], Tt[:, :, 1], Tt[:, :, 2])
                        nc.gpsimd.tensor_sub(Yp[:, 1], tY[:], Tt[:, :, 3])
                        # BN + ReLU: one activation per (ey, ex) quadrant,
                        # reading [n, ty, tx] from Y and writing the strided
                        # interleave into ost
                        Ypv = Yp[:].rearrange("p a b (t s) n -> p a b t s n",
                                              s=T7)
                        for ey in range(2):
                            for ex in range(2):
                                src = Ypv[:, ey, ex].transpose([0, 3, 1, 2])
                                dst = ost[:, :, ey::2, ex::2]
                                nc.scalar.activation(
                                    dst, src, AF.Relu,
                                    bias=bias_e[:, oc : oc + 1],
                                    scale=scale_e[:, oc : oc + 1])
                        ost_f = ost[:].rearrange("p n i j -> p n (i j)")
                        nsplit = 4 if last and oc == 1 else 2
                        nb_s = BATCH // nsplit
                        for h in range(nsplit):
                            ns = slice(h * nb_s, h * nb_s + nb_s)
                            nc.sync.dma_start(
                                out_v[:, oc, n0 + h * nb_s : n0 + (h + 1) * nb_s],
                                ost_f[:, ns])
                        # next batch's operands, emitted mid-batch so DVE/Pool
                        # produce them while the PE runs this batch's passes
                        if oc == 0 and b + 1 < NBATCH:
                            if b == 0:
                                box_math(8, 24, nc.vector)
                                mm_build(8, 24)
                            elif b == 2:
                                box_math(32, 32, nc.vector)
                            elif b == 3:
                                mm_build(32, 32)
                            nc.sync.dma_start(
                                XWb[(b + 1) % 2][:].rearrange(
                                    "p c w t s n -> p (c w t s n)"),
                                xwm_d[:, (b + 1) * XW2SZ : (b + 2) * XW2SZ])
                            g_build(b + 1)

    nc.compile()
    return nc


# ---------------------------------------------------------------------------
# host-side sharding / unsharding
# ---------------------------------------------------------------------------

def _xwm_host(mask):
    """2-D winograd transform of mask features on host: [NB,256,14,14] ->
    [128 cp, NBATCH, 2 ci, 16 uv, 7 ty, 7 tx, 8 n] bf16, flattened."""
    import ml_dtypes
    v = mask.reshape(NB, 2, 128, P, P)
    vp = np.zeros((NB, 2, 128, 16, 16), np.float32)
    vp[:, :, :, 1:15, 1:15] = v
    sw = np.lib.stride_tricks.sliding_window_view(vp, (4, 4), axis=(3, 4))
    til = sw[:, :, :, ::2, ::2]                    # [n, ci, cp, ty, tx, a, b]
    V = np.einsum('ua,vb,ncpyxab->pncuvyx', BT, BT, til,
                  optimize=True)                   # [cp, n, ci, u, v, ty, tx]
    V = V.reshape(128, NBATCH, BATCH, 2, UV, T7, T7)
    V = V.transpose(0, 1, 3, 4, 5, 6, 2)           # [cp, b, ci, uv, ty, tx, j]
    return np.ascontiguousarray(V.reshape(128, -1)).astype(ml_dtypes.bfloat16)


def _prep_in_maps(features, proposal_boxes, mask_features, conv_w, conv_b,
                  bn_gamma, bn_beta, bn_mean, bn_var):
    features = np.asarray(features, dtype=np.float32)
    proposal_boxes = np.asarray(proposal_boxes, dtype=np.float32)
    mask_features = np.asarray(mask_features, dtype=np.float32)
    conv_w = np.asarray(conv_w, dtype=np.float32)
    import ml_dtypes
    # weight layout: 2-D winograd transform U = G g G^T per (cin, cout):
    # [cout=256, cin=512, 3, 3] -> [cin_par=128, cin_hi=4, uv(16), cout], bf16
    wf = conv_w.reshape(256, 4, 128, 3, 3)                 # [o, hi, par, dy, dx]
    ww = np.einsum('uy,vx,ohpyx->phuvo', GM, GM, wf)       # [par, hi, u, v, o]
    wt = np.ascontiguousarray(ww.reshape(128, 4, UV, 256)).astype(ml_dtypes.bfloat16)
    epi = np.stack([np.asarray(x, dtype=np.float32) for x in
                    (conv_b, bn_gamma, bn_beta, bn_mean, bn_var)])  # [5, 256]
    epi = np.ascontiguousarray(epi.reshape(5, 2, 128).transpose(2, 0, 1)).astype(np.float32)
    cp = _consts_p()
    cfc = _consts_f()

    in_maps = []
    for i in range(N_CORES):
        img = i // (N_CORES // 2)
        n0 = (i * NB) % 256
        fimg = features[img]
        fsl = fimg[:, np.stack([YS0, YS1], axis=1), :]          # [256, 14, 2, W]
        fsl = fsl[:, :, :, np.stack([XS0, XS1], axis=1)]        # [256, 14, 2, 14, 2]
        fsl = np.ascontiguousarray(
            fsl.reshape(2, 128, P * 2 * P * 2).transpose(1, 0, 2).reshape(128, -1))
        in_maps.append({
            "featsl": fsl,
            "boxes": np.ascontiguousarray(proposal_boxes[img, n0 : n0 + NB]),
            "xwm": _xwm_host(mask_features[i * NB : (i + 1) * NB]),
            "wt": wt,
            "epi": epi,
            "consts_p": cp,
            "consts_f": cfc,
        })
    return in_maps


_NC_CACHE = {}


def _get_nc():
    if "nc" not in _NC_CACHE:
        _NC_CACHE["nc"] = build_kernel()
    return _NC_CACHE["nc"]


def _install_ntff_shim():
    """antenv.axon_hooks is missing in this image; shim it so trace=True works."""
    try:
        import antenv
        if hasattr(antenv, "axon_hooks"):
            return
        from trn_agent_boot.trn_boot import _ntff_profile_via_ctypes
        mod = types.ModuleType("antenv.axon_hooks")
        _h = [None]
        mod.set_axon_ntff_profile_hook = lambda h: _h.__setitem__(0, h)
        mod.get_axon_ntff_profile_hook = lambda: _h[0]
        sys.modules["antenv.axon_hooks"] = mod
        antenv.axon_hooks = mod
        mod.set_axon_ntff_profile_hook(_ntff_profile_via_ctypes("/opt/axon/libaxon_pjrt.so"))
    except Exception:
        pass


def run(trace=False, tmpdir=None, **inputs):
    from concourse.bass_utils import run_bass_kernel_spmd

    if trace:
        _install_ntff_shim()
    nc = _get_nc()
    in_maps = _prep_in_maps(**inputs)
    res = run_bass_kernel_spmd(nc, in_maps, core_ids=list(range(N_CORES)),
                               trace=trace, tmpdir=tmpdir)
    out = np.concatenate([np.asarray(res.results[i]["out"]) for i in range(N_CORES)], axis=0)
    return out.astype(np.float32), res


def kernel(**inputs):
    out, _ = run(trace=False, **inputs)
    return out
